# revision 2
# baseline (speedup 1.0000x reference)
"""Distributed causal-attention block kernel for 8 TRN2 NeuronCores.

Reference computation (per batch):
    xn = LayerNorm(x) * ln_w + ln_b
    q,k,v = xn @ {Wq,Wk,Wv}.T          (16 heads, head_dim 64)
    attn = causal_softmax(q k^T / 8) v
    out  = x + attn @ Wo.T + bo

Sharding (8 cores): core = 4*b + g  (b = batch 0/1, g = group 0..3)
  - QKV column-sharded: core computes heads 4g..4g+3 only.
  - Attention fully local per core (its 4 heads, all 2048 tokens).
  - AllGather (groups [[0..3],[4..7]]) of the per-head attention outputs
    A^T [256, 2048] bf16 -> [1024, 2048] on every core.
  - Out-projection column-sharded: core computes output dims
    [256g, 256g+256) for all tokens -> disjoint [2048, 256] slices,
    host gather is a pure concat.
  - ln_w / attention scale folded into weights on host; ln_b/bo folded
    into per-projection biases (added on device via K=1 matmuls when
    nonzero).

Compute dtype: bf16 on the TensorEngine (f32 PSUM accumulation), f32
LayerNorm statistics, f32 softmax normalization.
"""

import numpy as np
import ml_dtypes

import concourse.bass as bass
import concourse.mybir as mybir
import concourse.tile as tile
from concourse import bacc
from concourse.bass_utils import run_bass_kernel_spmd

F32 = mybir.dt.float32
BF16 = mybir.dt.bfloat16

B = 2
T = 2048          # sequence length
D = 1024          # embed dim
NH = 16           # total heads
HD = 64           # head dim
SCALE = HD ** -0.5
LN_EPS = 1e-5
N_CORES = 8
H_LOC = 4         # heads per core
DHL = H_LOC * HD  # 256 local head dims
NTT = T // 128    # 16 token tiles
NCH = T // 512    # 4 token chunks
DK = D // 128     # 8 contraction tiles

MASK_VAL = -1e9


def build_graph(has_qkv_bias: bool, has_o_bias: bool):
    nc = bacc.Bacc(None, target_bir_lowering=False)

    x_d = nc.declare_dram_parameter("x", [T, D], F32, isOutput=False)
    wqkv_d = nc.declare_dram_parameter("wqkv", [D, 3 * DHL], BF16, isOutput=False)
    wo_d = nc.declare_dram_parameter("wo", [D, DHL], BF16, isOutput=False)
    bias_d = nc.declare_dram_parameter("biases", [1, 4 * DHL], F32, isOutput=False)
    mask_d = nc.declare_dram_parameter("mask", [128, 128], F32, isOutput=False)
    ident_d = nc.declare_dram_parameter("ident", [128, 128], BF16, isOutput=False)
    xres_d = nc.declare_dram_parameter("xres", [T, DHL], F32, isOutput=False)
    out_d = nc.declare_dram_parameter("out", [T, DHL], F32, isOutput=True)

    with tile.TileContext(nc) as tc:
        with (
            tc.tile_pool(name="singles", bufs=1) as singles,
            tc.tile_pool(name="xin", bufs=3) as xin,
            tc.tile_pool(name="small", bufs=4) as small,
            tc.tile_pool(name="pbuf", bufs=2) as pbuf,
            tc.tile_pool(name="bden", bufs=2) as bden,
            tc.tile_pool(name="yout", bufs=3) as yout,
            tc.tile_pool(name="ps_s", bufs=2, space="PSUM") as ps_s,
            tc.tile_pool(name="ps_mm", bufs=2, space="PSUM") as ps_mm,
            tc.tile_pool(name="ps_o", bufs=2, space="PSUM") as ps_o,
            tc.tile_pool(name="dram", bufs=1, space="DRAM") as dram,
        ):
            # ---- constants / weights -------------------------------------
            wqkv_sb = singles.tile([128, DK, 3 * DHL], BF16)
            nc.sync.dma_start(
                out=wqkv_sb[:],
                in_=wqkv_d[:, :].rearrange("(k p) w -> p k w", p=128),
            )
            wo_sb = singles.tile([128, DK, DHL], BF16)
            nc.sync.dma_start(
                out=wo_sb[:],
                in_=wo_d[:, :].rearrange("(k p) w -> p k w", p=128),
            )
            bias_sb = singles.tile([1, 4 * DHL], F32)
            nc.sync.dma_start(out=bias_sb[:], in_=bias_d[:, :])
            mask_sb = singles.tile([128, 128], F32)
            nc.sync.dma_start(out=mask_sb[:], in_=mask_d[:, :])
            ident_sb = singles.tile([128, 128], BF16)
            nc.sync.dma_start(out=ident_sb[:], in_=ident_d[:, :])

            ones_col = singles.tile([1, 128], BF16)
            nc.vector.memset(ones_col[:], 1.0)
            ones_row = singles.tile([1, 512], F32)
            nc.vector.memset(ones_row[:], 1.0)
            eps_t = singles.tile([128, 1], F32)
            nc.vector.memset(eps_t[:], LN_EPS)

            # persistent activations
            xnT = singles.tile([128, DK, T], BF16)       # xn transposed
            qt_sb = singles.tile([128, 2, T], BF16)      # Q^T (2 head-pairs)
            kt_sb = singles.tile([128, 2, T], BF16)      # K^T
            vbuf = singles.tile([128, NTT, H_LOC * (HD + 1)], BF16)  # V | ones
            at_sb = singles.tile([128, 2, T], BF16)      # local A^T
            atall = singles.tile([128, DK, T], BF16)     # gathered A^T

            # ones column of vbuf (col 64 of each 65-wide head block)
            for h in range(H_LOC):
                nc.gpsimd.memset(vbuf[:, :, h * 65 + 64: h * 65 + 65], 1.0)

            # ---- stage 1: LayerNorm + transpose --------------------------
            for t in range(NTT):
                x_t = xin.tile([128, D], F32)
                nc.sync.dma_start(out=x_t[:], in_=x_d[t * 128:(t + 1) * 128, :])
                stats = small.tile([128, 2, 6], F32)
                nc.vector.bn_stats(out=stats[:, 0, :], in_=x_t[:, 0:512])
                nc.vector.bn_stats(out=stats[:, 1, :], in_=x_t[:, 512:1024])
                mv = small.tile([128, 2], F32)
                nc.vector.bn_aggr(out=mv[:], in_=stats[:])
                std = small.tile([128, 1], F32)
                nc.scalar.activation(
                    out=std[:], in_=mv[:, 1:2],
                    func=mybir.ActivationFunctionType.Sqrt, bias=eps_t[:],
                )
                rs = small.tile([128, 1], F32)
                nc.vector.reciprocal(out=rs[:], in_=std[:])
                xn_t = xin.tile([128, D], BF16)
                nc.vector.tensor_scalar(
                    out=xn_t[:], in0=x_t[:], scalar1=mv[:, 0:1], scalar2=rs[:],
                    op0=mybir.AluOpType.subtract, op1=mybir.AluOpType.mult,
                )
                ps_tr = ps_mm.tile([128, DK, 128], BF16, tag="mm")
                for dk in range(DK):
                    nc.tensor.transpose(
                        ps_tr[:, dk, :], xn_t[:, dk * 128:(dk + 1) * 128],
                        ident_sb[:],
                    )
                nc.vector.tensor_copy(
                    out=xnT[:, :, t * 128:(t + 1) * 128], in_=ps_tr[:]
                )

            # ---- stage 2 + 3 interleaved per chunk ------------------------
            for c in range(NCH):
                cs = c * 512
                # Q^T / K^T for this chunk
                for which, dest in ((0, qt_sb), (1, kt_sb)):
                    for hp in range(2):
                        pq = ps_mm.tile([128, 512], F32, tag="mm")
                        off = which * DHL + hp * 128
                        if has_qkv_bias:
                            nc.tensor.matmul(
                                pq[:], bias_sb[0:1, off:off + 128],
                                ones_row[:], start=True, stop=False,
                            )
                        for dk in range(DK):
                            nc.tensor.matmul(
                                pq[:],
                                wqkv_sb[:, dk, off:off + 128],
                                xnT[:, dk, cs:cs + 512],
                                start=(dk == 0 and not has_qkv_bias),
                                stop=(dk == DK - 1),
                            )
                        nc.vector.tensor_copy(
                            out=dest[:, hp, cs:cs + 512], in_=pq[:]
                        )
                # V for the 4 token tiles of this chunk
                for tt in range(c * 4, c * 4 + 4):
                    pv = ps_mm.tile([128, 512], F32, tag="mm")
                    pvs = pv[:, 0:DHL]
                    if has_qkv_bias:
                        nc.tensor.matmul(
                            pvs, ones_col[:],
                            bias_sb[0:1, 2 * DHL:3 * DHL],
                            start=True, stop=False,
                        )
                    for dk in range(DK):
                        nc.tensor.matmul(
                            pvs,
                            xnT[:, dk, tt * 128:(tt + 1) * 128],
                            wqkv_sb[:, dk, 2 * DHL:3 * DHL],
                            start=(dk == 0 and not has_qkv_bias),
                            stop=(dk == DK - 1),
                        )
                    nc.vector.tensor_copy(
                        out=vbuf[:, tt, :].rearrange(
                            "p (h c2) -> p h c2", c2=HD + 1
                        )[:, :, 0:HD],
                        in_=pvs.rearrange("p (h d) -> p h d", d=HD),
                    )

                # attention for q-chunk c, all 4 local heads
                kmax = 4 * (c + 1)
                for h in range(H_LOC):
                    pa = h % 2
                    hp = h // 2
                    po = pa * 64
                    p_sb = pbuf.tile([128, NTT, 512], BF16, tag="p")
                    for grp in range(kmax // 2):
                        pss = ps_s.tile([128, 1024], F32, tag="s")
                        for j in range(2):
                            kt = grp * 2 + j
                            i = kt - 4 * c  # band index (>=0: diagonal band)
                            qlo = 128 * i if i > 0 else 0
                            nc.tensor.matmul(
                                pss[:, j * 512 + qlo: (j + 1) * 512],
                                kt_sb[po:po + 64, hp, kt * 128:(kt + 1) * 128],
                                qt_sb[po:po + 64, hp, cs + qlo: cs + 512],
                                start=True, stop=True,
                            )
                            if i >= 0:
                                nc.vector.tensor_tensor(
                                    out=pss[:, j * 512 + qlo: j * 512 + qlo + 128],
                                    in0=pss[:, j * 512 + qlo: j * 512 + qlo + 128],
                                    in1=mask_sb[:],
                                    op=mybir.AluOpType.add,
                                )
                        nc.scalar.activation(
                            out=p_sb[:, grp * 2: grp * 2 + 2, :],
                            in_=pss[:].rearrange("p (a b) -> p a b", a=2),
                            func=mybir.ActivationFunctionType.Exp,
                        )
                        for j in range(2):
                            kt = grp * 2 + j
                            i = kt - 4 * c
                            if i > 0:
                                nc.gpsimd.memset(p_sb[:, kt, 0:128 * i], 0.0)
                    poo = ps_o.tile([65, 512], F32, tag="o")
                    for kt in range(kmax):
                        nc.tensor.matmul(
                            poo[:],
                            vbuf[:, kt, h * 65: h * 65 + 65],
                            p_sb[:, kt, :],
                            start=(kt == 0), stop=(kt == kmax - 1),
                        )
                    den_r = small.tile([1, 512], BF16)
                    with nc.allow_low_precision(
                        reason="softmax denom reciprocal in bf16"
                    ):
                        nc.vector.reciprocal(out=den_r[:], in_=poo[64:65, :])
                    pb = ps_mm.tile([64, 512], F32, tag="mm")
                    nc.tensor.matmul(
                        pb[:], ones_col[0:1, 0:64], den_r[:],
                        start=True, stop=True,
                    )
                    b_sb = bden.tile([64, 512], F32)
                    nc.scalar.copy(out=b_sb[:], in_=pb[:])
                    nc.vector.tensor_tensor(
                        out=at_sb[po:po + 64, hp, cs:cs + 512],
                        in0=poo[0:64, :], in1=b_sb[:],
                        op=mybir.AluOpType.mult,
                    )

            # ---- stage 4: AllGather of A^T -------------------------------
            ag_in = dram.tile([DHL, T], BF16)
            ag_out = dram.tile([N_CORES // 2 * DHL, T], BF16)
            for i in range(2):
                nc.gpsimd.dma_start(
                    out=ag_in[i * 128:(i + 1) * 128, :], in_=at_sb[:, i, :]
                )
            nc.gpsimd.collective_compute(
                "AllGather",
                mybir.AluOpType.bypass,
                replica_groups=[[0, 1, 2, 3], [4, 5, 6, 7]],
                ins=[ag_in.opt()],
                outs=[ag_out.opt()],
            )
            nc.sync.dma_start(
                out=atall[:],
                in_=ag_out[:, :].rearrange("(k p) t -> p k t", p=128),
            )

            # ---- stage 5: out-projection + residual ----------------------
            for t in range(NTT):
                xr_t = yout.tile([128, DHL], F32, tag="xr")
                nc.sync.dma_start(
                    out=xr_t[:], in_=xres_d[t * 128:(t + 1) * 128, :]
                )
                py = ps_s.tile([128, DHL], F32, tag="s")
                if has_o_bias:
                    nc.tensor.matmul(
                        py[:], ones_col[:], bias_sb[0:1, 3 * DHL:4 * DHL],
                        start=True, stop=False,
                    )
                for kk in range(DK):
                    nc.tensor.matmul(
                        py[:],
                        atall[:, kk, t * 128:(t + 1) * 128],
                        wo_sb[:, kk, :],
                        start=(kk == 0 and not has_o_bias),
                        stop=(kk == DK - 1),
                    )
                y_sb = yout.tile([128, DHL], F32, tag="y")
                nc.vector.tensor_tensor(
                    out=y_sb[:], in0=py[:], in1=xr_t[:],
                    op=mybir.AluOpType.add,
                )
                nc.sync.dma_start(
                    out=out_d[t * 128:(t + 1) * 128, :], in_=y_sb[:]
                )

    nc.compile()
    return nc


_graph_cache = {}


def _get_graph(has_qkv_bias, has_o_bias):
    key = (has_qkv_bias, has_o_bias)
    if key not in _graph_cache:
        _graph_cache[key] = build_graph(*key)
    return _graph_cache[key]


def _bf16(a):
    return np.ascontiguousarray(a.astype(ml_dtypes.bfloat16))


def kernel(x, ln_w, ln_b, Wq, Wk, Wv, Wo, bo, _want_trace=False):
    x = np.asarray(x, dtype=np.float32)
    ln_w = np.asarray(ln_w, dtype=np.float32)
    ln_b = np.asarray(ln_b, dtype=np.float32)
    Wq = np.asarray(Wq, dtype=np.float32)
    Wk = np.asarray(Wk, dtype=np.float32)
    Wv = np.asarray(Wv, dtype=np.float32)
    Wo = np.asarray(Wo, dtype=np.float32)
    bo = np.asarray(bo, dtype=np.float32)

    mask = np.where(
        np.arange(128)[:, None] <= np.arange(128)[None, :], 0.0, MASK_VAL
    ).astype(np.float32)
    ident = np.eye(128, dtype=ml_dtypes.bfloat16)

    bq_all = (Wq @ ln_b) * SCALE
    bk_all = Wk @ ln_b
    bv_all = Wv @ ln_b
    has_qkv_bias = bool(
        np.abs(bq_all).max() > 0 or np.abs(bk_all).max() > 0
        or np.abs(bv_all).max() > 0
    )
    has_o_bias = bool(np.abs(bo).max() > 0)

    in_maps = []
    for core in range(N_CORES):
        b, g = divmod(core, 4)
        hs = g * DHL
        wq_s = (Wq[hs:hs + DHL, :] * ln_w[None, :]).T * SCALE
        wk_s = (Wk[hs:hs + DHL, :] * ln_w[None, :]).T
        wv_s = (Wv[hs:hs + DHL, :] * ln_w[None, :]).T
        wqkv = _bf16(np.concatenate([wq_s, wk_s, wv_s], axis=1))
        wo_s = _bf16(Wo[hs:hs + DHL, :].T)
        biases = np.concatenate(
            [bq_all[hs:hs + DHL], bk_all[hs:hs + DHL], bv_all[hs:hs + DHL],
             bo[hs:hs + DHL]]
        ).astype(np.float32)[None, :]
        in_maps.append({
            "x": np.ascontiguousarray(x[b]),
            "wqkv": wqkv,
            "wo": wo_s,
            "biases": np.ascontiguousarray(biases),
            "mask": mask,
            "ident": ident,
            "xres": np.ascontiguousarray(x[b][:, hs:hs + DHL]),
        })

    nc = _get_graph(has_qkv_bias, has_o_bias)
    res = run_bass_kernel_spmd(
        nc, in_maps, core_ids=list(range(N_CORES)), trace=_want_trace
    )

    out = np.empty((B, T, D), dtype=np.float32)
    for core in range(N_CORES):
        b, g = divmod(core, 4)
        out[b, :, g * DHL:(g + 1) * DHL] = res.results[core]["out"]
    if _want_trace:
        kernel.last_results = res
    return out
